# revision 9
# baseline (speedup 1.0000x reference)
"""Trainium2 Bass kernel v3 for 3rd-order HONU (nn_HONU_80865644249720).

out[b] = sum_{i<=j<=k} w_{ijk} xb_i xb_j xb_k,  xb = [1, x] (65 feats)

Pair-squaring with x^2 corrections folded into pair weights (as v2), plus:
  - first input blob DMA'd PRE-TileContext: issues right after the init
    barrier and keeps PE SEQ occupied, so sel matmuls run at full p-state
  - zt split into two PSUM accumulators (tiles 0-3 / 4-7); the first
    half's copy+DMA overlaps the second half's squares, so only ztB pays
    the full DMA tail
  - square stage rebalanced across ACT (direct Square, optionally on
    merged [128,1024] PSUM double-tiles) / DVE (copy+mul) / Pool (mul)

Per core (4-way batch x 2-way pairs, BC=512):
  sel_t : ps[128p,512b] = es_t^T @ xbt           (PE, bf16, 8x, 213ns)
  sq_t  : ss = ps^2 -> bf16 SBUF                 (ACT/DVE/Pool per SQ_PLAN)
  wf_t,c: zt[128b,65] += ss[:,c128]^T @ wh_t     (PE, bf16, 32x small)
  out   : ztA/ztB -> bf16 SBUF -> DMA -> DRAM [256,260]
Host: bias pairs, 96 residual pairs, final contraction with xb.
"""

import os
from contextlib import ExitStack

import numpy as np

IN_FEATURES = 64
NF = IN_FEATURES + 1  # 65 features incl. bias
BATCH = 2048
N_CORES = 8
NBS = 4  # batch shards
NPS = 2  # pair shards
BC = BATCH // NBS  # 512 batch rows per core
PH = 8  # pair tiles per half
PCOLS = PH * 128  # 1024 pair columns per half
NCH = 4  # batch chunks for the flipped W matmuls
CHB = BC // NCH  # 128
OUTW = NCH * NF  # 260

# es tiles in blob a1 (with xbt) vs blob a2
ES_A1 = 3

# square plan: per tile one of
#   'a' ACT single Square
#   'm' ACT merged Square over this tile + next (same PSUM double tile;
#       must be at even t; next slot is 'x')
#   'v' DVE copy + DVE mul
#   'g' DVE copy + Pool mul
#   'q' ACT copy + Pool mul
#   'h' split: ACT squares first half, DVE copy+mul second half
SQ_PLAN = "aaaaaaaa"
# group split: tiles [0:GSPLIT) -> ztA, rest -> ztB
GSPLIT = 4
# zt copy engines: 'v' DVE, 'a' ACT, 'p' Pool
CP_A = "v"
CP_B = "v"

_CACHE = {}
LAST_RESULTS = None


def _build_bass():
    import concourse.bacc as bacc
    import concourse.mybir as mybir
    import concourse.tile as tile

    f32 = mybir.dt.float32
    bf16 = mybir.dt.float16
    Square = mybir.ActivationFunctionType.Square
    Copy = mybir.ActivationFunctionType.Copy

    nc = bacc.Bacc(
        target_bir_lowering=False,
        debug=False,
        enable_asserts=False,
        num_devices=N_CORES,
    )

    A1C = BC + ES_A1 * 128
    A2C = (PH - ES_A1) * 128
    a1_d = nc.dram_tensor("a1", [64, A1C], bf16, kind="ExternalInput").ap()
    a2_d = nc.dram_tensor("a2", [64, A2C], bf16, kind="ExternalInput").ap()
    c_d = nc.dram_tensor("c", [128, PH * NF], bf16, kind="ExternalInput").ap()
    out_d = nc.dram_tensor("out", [256, OUTW], bf16, kind="ExternalOutput").ap()

    ctx0 = ExitStack()
    a1_sb = ctx0.enter_context(nc.sbuf_tensor([64, A1C], bf16))
    sem_a1 = nc.alloc_semaphore("in_a1")
    nc.sync.dma_start(a1_sb.ap(), a1_d).then_inc(sem_a1, 16)
    nc.tensor.wait_ge(sem_a1, 16)
    a1 = a1_sb.ap()

    with tile.TileContext(nc) as tc, ExitStack() as ctx:
        consts = ctx.enter_context(tc.tile_pool(name="consts", bufs=1))
        ss_pool = ctx.enter_context(tc.tile_pool(name="ss", bufs=9))
        psc_pool = ctx.enter_context(tc.tile_pool(name="psc", bufs=5))
        ps_pool = ctx.enter_context(tc.tile_pool(name="ps", bufs=3, space="PSUM"))
        zt_pool = ctx.enter_context(tc.tile_pool(name="zt", bufs=1, space="PSUM"))

        a2 = consts.tile([64, A2C], bf16, tag="a2")
        c = consts.tile([128, PH * NF], bf16, tag="c")
        ztA_sb = consts.tile([128, OUTW], bf16, tag="ztA_sb")
        ztB_sb = consts.tile([128, OUTW], bf16, tag="ztB_sb")

        nc.sync.dma_start(a2[:], a2_d)
        nc.scalar.dma_start(c[:], c_d)

        xbt = a1[:, 0:BC]

        def es_tile(t):
            if t < ES_A1:
                return a1[:, BC + t * 128 : BC + (t + 1) * 128]
            return a2[:, (t - ES_A1) * 128 : (t - ES_A1 + 1) * 128]

        # PSUM double tiles: allocated per pair through the pool so Tile
        # tracks buffer-reuse dependencies (bufs=3 -> 3 live doubles)
        doubles = [None] * ((PH + 1) // 2)

        zts = []
        for g in range(2):
            zt_ = zt_pool.tile([128, OUTW], f32, name=f"zt{g}")
            zts.append(zt_)

        # --- sels first: dense PE stream at full p-state ---
        ps_aps = []
        for t in range(PH):
            if t % 2 == 0:
                d_ = ps_pool.tile([128, 1024], f32, tag="psd", name=f"psd{t}")
                doubles[t // 2] = d_
            d_ = doubles[t // 2]
            ps = d_[:, (t % 2) * BC : (t % 2 + 1) * BC]
            nc.tensor.matmul(ps, es_tile(t), xbt)
            ps_aps.append(ps)

        def ps_slot(t):
            d_ = doubles[t // 2]
            return d_, ps_aps[t]

        ss_tiles = [None] * PH  # (base_ap, col_offset)

        def emit_sq(t):
            plan = SQ_PLAN[t]
            ps = ps_aps[t]
            if plan == "m":
                dtile, _ = ps_slot(t)
                ss = ss_pool.tile([128, 1024], bf16, tag="ssm")
                nc.scalar.activation(ss[:], dtile[:], Square)
                ss_tiles[t] = (ss, 0)
                ss_tiles[t + 1] = (ss, BC)
                return
            ss = ss_pool.tile([128, BC], bf16, tag="ss")
            if plan == "a":
                nc.scalar.activation(ss[:], ps, Square)
            elif plan == "h":
                hb = BC // 2
                nc.scalar.activation(ss[:, 0:hb], ps[:, 0:hb], Square)
                psc = psc_pool.tile([128, hb], bf16, tag="psch")
                nc.vector.tensor_copy(psc[:], ps[:, hb:])
                nc.vector.tensor_mul(ss[:, hb:], psc[:], psc[:])
            else:
                psc = psc_pool.tile([128, BC], bf16, tag="psc")
                if plan == "q":
                    nc.scalar.activation(psc[:], ps, Copy)
                else:
                    nc.vector.tensor_copy(psc[:], ps)
                if plan in ("g", "q"):
                    nc.gpsimd.tensor_mul(ss[:], psc[:], psc[:])
                else:
                    nc.vector.tensor_mul(ss[:], psc[:], psc[:])
            ss_tiles[t] = (ss, 0)

        def emit_wf(t, zt, start, stop):
            ss, off = ss_tiles[t]
            for ch in range(NCH):
                nc.tensor.matmul(
                    zt[:, ch * NF : (ch + 1) * NF],
                    ss[:, off + ch * CHB : off + (ch + 1) * CHB],
                    c[:, t * NF : (t + 1) * NF],
                    start=start and ch == 0,
                    stop=stop,
                    skip_group_check=True,
                )

        for t in range(PH):
            if SQ_PLAN[t] != "x":
                emit_sq(t)

        def emit_copy(eng, dst, src):
            if eng == "v":
                nc.vector.tensor_copy(dst, src)
            elif eng == "a":
                nc.scalar.activation(dst, src, Copy)
            else:
                nc.gpsimd.tensor_copy(dst, src)

        groups = [list(range(0, GSPLIT)), list(range(GSPLIT, PH))]
        for g, tiles in enumerate(groups):
            for t in tiles:
                emit_wf(t, zts[g], start=(t == tiles[0]), stop=(t == tiles[-1]))
            if g == 0:
                emit_copy(CP_A, ztA_sb[:], zts[0][:])
                nc.sync.dma_start(out_d[0:128, :], ztA_sb[:])
        emit_copy(CP_B, ztB_sb[:], zts[1][:])
        nc.sync.dma_start(out_d[128:256, :], ztB_sb[:])

    nc.compile()
    _CACHE["ctx0"] = ctx0
    return nc


def _pair_maps():
    jp = np.concatenate([np.full(NF - j, j, np.int64) for j in range(NF)])
    kp = np.concatenate([np.arange(j, NF, dtype=np.int64) for j in range(NF)])
    return jp, kp


def _host_prep(x, weights, comb_idx):
    """Build per-core bf16 input blobs (numpy only)."""
    bf16 = np.float16
    jp, kp = _pair_maps()
    npair = len(jp)  # 2145

    ci = np.asarray(comb_idx, np.int64)
    c0, c1, c2 = ci[:, 0], ci[:, 1], ci[:, 2]
    pcol = c1 * NF - (c1 * (c1 - 1)) // 2 + (c2 - c1)
    w2 = np.zeros((npair, NF), np.float32)  # [pair, i]
    w2[pcol, c0] = np.asarray(weights, np.float32)

    xb = np.concatenate(
        [np.ones((BATCH, 1), np.float32), np.asarray(x, np.float32)], axis=1
    )

    # host-side terms: bias pairs (0,k) carry only linear/const monomials;
    # plus residual off-diag pairs so each half is exactly PCOLS.
    bias_pairs = np.where(jp == 0)[0]
    host_out = (xb @ w2[bias_pairs, 0]).astype(np.float64)
    dev = np.where(jp >= 1)[0]
    diag_idx = dev[jp[dev] == kp[dev]]  # 64 diag pairs, feature order
    off_idx = dev[jp[dev] != kp[dev]]  # 2016
    noff = PCOLS - 64  # off-diag pairs per half on device
    offA, offB = off_idx[:1008], off_idx[1008:]
    nres = 1008 - noff  # residual pairs per half -> host
    rem = np.concatenate([offA[1008 - nres :], offB[1008 - nres :]])
    wps = xb @ w2[rem].T
    host_out += np.sum(
        wps * xb[:, jp[rem]] * xb[:, kp[rem]], axis=1, dtype=np.float64
    )
    off_h = [offA[:noff], offB[:noff]]

    es_h, wh_h = [], []
    for h in range(NPS):
        offs = off_h[h]
        pidx = np.concatenate([diag_idx, offs])
        pcount = len(pidx)  # PCOLS exactly
        es = np.zeros((64, PCOLS), np.float32)  # feature f -> row f-1
        np.add.at(es, (jp[pidx] - 1, np.arange(pcount)), 1.0)
        np.add.at(es, (kp[pidx] - 1, np.arange(pcount)), 1.0)
        wh = np.zeros((PCOLS, NF), np.float32)
        wh[64:pcount] = 0.5 * w2[offs]
        if h == 0:
            wh[:64] = 0.25 * w2[diag_idx]  # ss_diag = 4 x^2
        np.add.at(wh, jp[offs] - 1, -0.125 * w2[offs])
        np.add.at(wh, kp[offs] - 1, -0.125 * w2[offs])
        es_h.append(es.astype(bf16))
        # wh tiles: [128, PH*NF], tile t = wh[t*128:(t+1)*128, :]
        wh_h.append(
            np.ascontiguousarray(
                wh.reshape(PH, 128, NF).transpose(1, 0, 2).reshape(128, PH * NF)
            ).astype(bf16)
        )

    xbt_q = []
    for q in range(NBS):
        xbt_q.append(
            np.ascontiguousarray(xb[q * BC : (q + 1) * BC, 1:].T).astype(bf16)
        )

    in_maps = []
    for core in range(N_CORES):
        q, h = core % NBS, core // NBS
        a1_blob = np.concatenate([xbt_q[q], es_h[h][:, 0 : ES_A1 * 128]], axis=1)
        a2_blob = np.ascontiguousarray(es_h[h][:, ES_A1 * 128 :])
        in_maps.append(
            {
                "a1": np.ascontiguousarray(a1_blob),
                "a2": a2_blob,
                "c": wh_h[h],
            }
        )
    return in_maps, xb, host_out


def kernel(x, weights, comb_idx):
    global LAST_RESULTS
    from concourse import bass_utils

    if "nc" not in _CACHE:
        _CACHE["nc"] = _build_bass()
    nc = _CACHE["nc"]

    in_maps, xb, host_out = _host_prep(x, weights, comb_idx)
    res = bass_utils.run_bass_kernel_spmd(
        nc,
        in_maps,
        core_ids=list(range(N_CORES)),
        trace=bool(int(os.environ.get("HONU_TRACE", "0"))),
    )
    LAST_RESULTS = res

    out = host_out.copy()
    for core in range(N_CORES):
        q = core % NBS
        blob = np.asarray(res.results[core]["out"], np.float64)  # [256, OUTW]
        zt = (blob[0:128] + blob[128:256]).reshape(128, NCH, NF)
        for ch in range(NCH):
            rows = slice(q * BC + ch * CHB, q * BC + (ch + 1) * CHB)
            out[rows] += np.sum(zt[:, ch, :] * xb[rows], axis=1)
    return out.reshape(BATCH, 1).astype(np.float32)


# revision 16
# speedup vs baseline: 1.0748x; 1.0748x over previous
"""Trainium2 Bass kernel v3 for 3rd-order HONU (nn_HONU_80865644249720).

out[b] = sum_{i<=j<=k} w_{ijk} xb_i xb_j xb_k,  xb = [1, x] (65 feats)

Pair-squaring with x^2 corrections folded into pair weights (as v2), plus:
  - first input blob DMA'd PRE-TileContext: issues right after the init
    barrier and keeps PE SEQ occupied, so sel matmuls run at full p-state
  - zt split into two PSUM accumulators (tiles 0-3 / 4-7); the first
    half's copy+DMA overlaps the second half's squares, so only ztB pays
    the full DMA tail
  - square stage rebalanced across ACT (direct Square, optionally on
    merged [128,1024] PSUM double-tiles) / DVE (copy+mul) / Pool (mul)

Per core (4-way batch x 2-way pairs, BC=512):
  sel_t : ps[128p,512b] = es_t^T @ xbt           (PE, bf16, 8x, 213ns)
  sq_t  : ss = ps^2 -> bf16 SBUF                 (ACT/DVE/Pool per SQ_PLAN)
  wf_t,c: zt[128b,65] += ss[:,c128]^T @ wh_t     (PE, bf16, 32x small)
  out   : ztA/ztB -> bf16 SBUF -> DMA -> DRAM [256,260]
Host: bias pairs, 96 residual pairs, final contraction with xb.
"""

import os
from contextlib import ExitStack

import numpy as np

IN_FEATURES = 64
NF = IN_FEATURES + 1  # 65 features incl. bias
BATCH = 2048
N_CORES = 8
NBS = 4  # batch shards
NPS = 2  # pair shards
BC = BATCH // NBS  # 512 batch rows per core
PH = 8  # pair tiles per half
PCOLS = PH * 128  # 1024 pair columns per half
NCH = 4  # batch chunks for the flipped W matmuls
CHB = BC // NCH  # 128
OUTW = NCH * NF  # 260

# es tiles in blob a1 (with xbt) vs blob a2
ES_A1 = 3

# square plan: per tile one of
#   'a' ACT single Square
#   'm' ACT merged Square over this tile + next (same PSUM double tile;
#       must be at even t; next slot is 'x')
#   'v' DVE copy + DVE mul
#   'g' DVE copy + Pool mul
#   'q' ACT copy + Pool mul
#   'd' ACT copy + DVE mul
#   'h' split: ACT squares first half, DVE copy+mul second half
# CONFIG is overridable (the tuner rewrites it before _build_bass)
CONFIG = {
    "SQ_PLAN": "mxvgmxah",
    # wf group -> ztA / ztB (position lists; A's copy+DMA overlap B)
    "GROUP_A": (0, 1, 2, 3),
    "GROUP_B": (4, 5, 6, 7),
    # zt copy engines: 'v' DVE, 'a' ACT, 'p' Pool
    "CP_A": "v",
    "CP_B": "v",
}

_CACHE = {}
LAST_RESULTS = None


def _build_bass():
    import concourse.bacc as bacc
    import concourse.mybir as mybir
    import concourse.tile as tile

    f32 = mybir.dt.float32
    bf16 = mybir.dt.float16
    Square = mybir.ActivationFunctionType.Square
    Copy = mybir.ActivationFunctionType.Copy

    nc = bacc.Bacc(
        target_bir_lowering=False,
        debug=False,
        enable_asserts=False,
        num_devices=N_CORES,
    )

    A1C = BC + ES_A1 * 128
    A2C = (PH - ES_A1) * 128
    a1_d = nc.dram_tensor("a1", [64, A1C], bf16, kind="ExternalInput").ap()
    a2_d = nc.dram_tensor("a2", [64, A2C], bf16, kind="ExternalInput").ap()
    c_d = nc.dram_tensor("c", [128, PH * NF], bf16, kind="ExternalInput").ap()
    out_d = nc.dram_tensor("out", [256, OUTW], bf16, kind="ExternalOutput").ap()

    ctx0 = ExitStack()
    a1_sb = ctx0.enter_context(nc.sbuf_tensor([64, A1C], bf16))
    sem_a1 = nc.alloc_semaphore("in_a1")
    nc.sync.dma_start(a1_sb.ap(), a1_d).then_inc(sem_a1, 16)
    nc.tensor.wait_ge(sem_a1, 16)
    a1 = a1_sb.ap()

    with tile.TileContext(nc) as tc, ExitStack() as ctx:
        consts = ctx.enter_context(tc.tile_pool(name="consts", bufs=1))
        ss_pool = ctx.enter_context(tc.tile_pool(name="ss", bufs=9))
        psc_pool = ctx.enter_context(tc.tile_pool(name="psc", bufs=5))
        ps_pool = ctx.enter_context(tc.tile_pool(name="ps", bufs=3, space="PSUM"))
        zt_pool = ctx.enter_context(tc.tile_pool(name="zt", bufs=1, space="PSUM"))

        a2 = consts.tile([64, A2C], bf16, tag="a2")
        c = consts.tile([128, PH * NF], bf16, tag="c")
        ztA_sb = consts.tile([128, OUTW], bf16, tag="ztA_sb")
        ztB_sb = consts.tile([128, OUTW], bf16, tag="ztB_sb")

        nc.sync.dma_start(a2[:], a2_d)
        nc.sync.dma_start(c[:], c_d)

        xbt = a1[:, 0:BC]

        def es_tile(t):
            if t < ES_A1:
                return a1[:, BC + t * 128 : BC + (t + 1) * 128]
            return a2[:, (t - ES_A1) * 128 : (t - ES_A1 + 1) * 128]

        # PSUM double tiles: allocated per pair through the pool so Tile
        # tracks buffer-reuse dependencies (bufs=3 -> 3 live doubles)
        doubles = [None] * ((PH + 1) // 2)

        zts = []
        for g in range(2):
            zt_ = zt_pool.tile([128, OUTW], f32, name=f"zt{g}")
            zts.append(zt_)

        # --- sels first: dense PE stream at full p-state ---
        ps_aps = []
        for t in range(PH):
            if t % 2 == 0:
                d_ = ps_pool.tile([128, 1024], f32, tag="psd", name=f"psd{t}")
                doubles[t // 2] = d_
            d_ = doubles[t // 2]
            ps = d_[:, (t % 2) * BC : (t % 2 + 1) * BC]
            nc.tensor.matmul(ps, es_tile(t), xbt)
            ps_aps.append(ps)

        def ps_slot(t):
            d_ = doubles[t // 2]
            return d_, ps_aps[t]

        ss_tiles = [None] * PH  # (base_ap, col_offset)

        SQ_PLAN = CONFIG["SQ_PLAN"]

        def emit_sq(t):
            plan = SQ_PLAN[t]
            ps = ps_aps[t]
            if plan == "m":
                dtile, _ = ps_slot(t)
                ss = ss_pool.tile([128, 1024], bf16, tag="ssm")
                nc.scalar.activation(ss[:], dtile[:], Square)
                ss_tiles[t] = (ss, 0)
                ss_tiles[t + 1] = (ss, BC)
                return
            ss = ss_pool.tile([128, BC], bf16, tag="ss")
            if plan == "a":
                nc.scalar.activation(ss[:], ps, Square)
            elif plan == "h":
                hb = BC // 2
                nc.scalar.activation(ss[:, 0:hb], ps[:, 0:hb], Square)
                psc = psc_pool.tile([128, hb], bf16, tag="psch")
                nc.vector.tensor_copy(psc[:], ps[:, hb:])
                nc.vector.tensor_mul(ss[:, hb:], psc[:], psc[:])
            else:
                psc = psc_pool.tile([128, BC], bf16, tag="psc")
                if plan in ("q", "d"):
                    nc.scalar.activation(psc[:], ps, Copy)
                else:
                    nc.vector.tensor_copy(psc[:], ps)
                if plan in ("g", "q"):
                    nc.gpsimd.tensor_mul(ss[:], psc[:], psc[:])
                else:
                    nc.vector.tensor_mul(ss[:], psc[:], psc[:])
            ss_tiles[t] = (ss, 0)

        def emit_wf(t, zt, start, stop):
            ss, off = ss_tiles[t]
            for ch in range(NCH):
                nc.tensor.matmul(
                    zt[:, ch * NF : (ch + 1) * NF],
                    ss[:, off + ch * CHB : off + (ch + 1) * CHB],
                    c[:, t * NF : (t + 1) * NF],
                    start=start and ch == 0,
                    stop=stop,
                    skip_group_check=True,
                )

        for t in range(PH):
            if SQ_PLAN[t] != "x":
                emit_sq(t)

        def emit_copy(eng, dst, src):
            if eng == "v":
                nc.vector.tensor_copy(dst, src)
            elif eng == "a":
                nc.scalar.activation(dst, src, Copy)
            else:
                nc.gpsimd.tensor_copy(dst, src)

        groups = [list(CONFIG["GROUP_A"]), list(CONFIG["GROUP_B"])]
        for g, tiles in enumerate(groups):
            for t in tiles:
                emit_wf(t, zts[g], start=(t == tiles[0]), stop=(t == tiles[-1]))
            if g == 0:
                emit_copy(CONFIG["CP_A"], ztA_sb[:], zts[0][:])
                nc.sync.dma_start(out_d[0:128, :], ztA_sb[:])
        emit_copy(CONFIG["CP_B"], ztB_sb[:], zts[1][:])
        nc.sync.dma_start(out_d[128:256, :], ztB_sb[:])

    nc.compile()
    _CACHE["ctx0"] = ctx0
    return nc


def _pair_maps():
    jp = np.concatenate([np.full(NF - j, j, np.int64) for j in range(NF)])
    kp = np.concatenate([np.arange(j, NF, dtype=np.int64) for j in range(NF)])
    return jp, kp


def _host_prep(x, weights, comb_idx):
    """Build per-core bf16 input blobs (numpy only)."""
    bf16 = np.float16
    jp, kp = _pair_maps()
    npair = len(jp)  # 2145

    ci = np.asarray(comb_idx, np.int64)
    c0, c1, c2 = ci[:, 0], ci[:, 1], ci[:, 2]
    pcol = c1 * NF - (c1 * (c1 - 1)) // 2 + (c2 - c1)
    w2 = np.zeros((npair, NF), np.float32)  # [pair, i]
    w2[pcol, c0] = np.asarray(weights, np.float32)

    xb = np.concatenate(
        [np.ones((BATCH, 1), np.float32), np.asarray(x, np.float32)], axis=1
    )

    # host-side terms: bias pairs (0,k) carry only linear/const monomials;
    # plus residual off-diag pairs so each half is exactly PCOLS.
    bias_pairs = np.where(jp == 0)[0]
    host_out = (xb @ w2[bias_pairs, 0]).astype(np.float64)
    dev = np.where(jp >= 1)[0]
    diag_idx = dev[jp[dev] == kp[dev]]  # 64 diag pairs, feature order
    off_idx = dev[jp[dev] != kp[dev]]  # 2016
    noff = PCOLS - 64  # off-diag pairs per half on device
    offA, offB = off_idx[:1008], off_idx[1008:]
    nres = 1008 - noff  # residual pairs per half -> host
    rem = np.concatenate([offA[1008 - nres :], offB[1008 - nres :]])
    wps = xb @ w2[rem].T
    host_out += np.sum(
        wps * xb[:, jp[rem]] * xb[:, kp[rem]], axis=1, dtype=np.float64
    )
    off_h = [offA[:noff], offB[:noff]]

    es_h, wh_h = [], []
    for h in range(NPS):
        offs = off_h[h]
        pidx = np.concatenate([diag_idx, offs])
        pcount = len(pidx)  # PCOLS exactly
        es = np.zeros((64, PCOLS), np.float32)  # feature f -> row f-1
        np.add.at(es, (jp[pidx] - 1, np.arange(pcount)), 1.0)
        np.add.at(es, (kp[pidx] - 1, np.arange(pcount)), 1.0)
        wh = np.zeros((PCOLS, NF), np.float32)
        wh[64:pcount] = 0.5 * w2[offs]
        if h == 0:
            wh[:64] = 0.25 * w2[diag_idx]  # ss_diag = 4 x^2
        np.add.at(wh, jp[offs] - 1, -0.125 * w2[offs])
        np.add.at(wh, kp[offs] - 1, -0.125 * w2[offs])
        es_h.append(es.astype(bf16))
        # wh tiles: [128, PH*NF], tile t = wh[t*128:(t+1)*128, :]
        wh_h.append(
            np.ascontiguousarray(
                wh.reshape(PH, 128, NF).transpose(1, 0, 2).reshape(128, PH * NF)
            ).astype(bf16)
        )

    xbt_q = []
    for q in range(NBS):
        xbt_q.append(
            np.ascontiguousarray(xb[q * BC : (q + 1) * BC, 1:].T).astype(bf16)
        )

    in_maps = []
    for core in range(N_CORES):
        q, h = core % NBS, core // NBS
        a1_blob = np.concatenate([xbt_q[q], es_h[h][:, 0 : ES_A1 * 128]], axis=1)
        a2_blob = np.ascontiguousarray(es_h[h][:, ES_A1 * 128 :])
        in_maps.append(
            {
                "a1": np.ascontiguousarray(a1_blob),
                "a2": a2_blob,
                "c": wh_h[h],
            }
        )
    return in_maps, xb, host_out


def kernel(x, weights, comb_idx):
    global LAST_RESULTS
    from concourse import bass_utils

    if "nc" not in _CACHE:
        _CACHE["nc"] = _build_bass()
    nc = _CACHE["nc"]

    in_maps, xb, host_out = _host_prep(x, weights, comb_idx)
    res = bass_utils.run_bass_kernel_spmd(
        nc,
        in_maps,
        core_ids=list(range(N_CORES)),
        trace=bool(int(os.environ.get("HONU_TRACE", "0"))),
    )
    LAST_RESULTS = res

    out = host_out.copy()
    for core in range(N_CORES):
        q = core % NBS
        blob = np.asarray(res.results[core]["out"], np.float64)  # [256, OUTW]
        zt = (blob[0:128] + blob[128:256]).reshape(128, NCH, NF)
        for ch in range(NCH):
            rows = slice(q * BC + ch * CHB, q * BC + (ch + 1) * CHB)
            out[rows] += np.sum(zt[:, ch, :] * xb[rows], axis=1)
    return out.reshape(BATCH, 1).astype(np.float32)
